# revision 14
# baseline (speedup 1.0000x reference)
"""AGNNConv on 8 Trainium2 NeuronCores — dense matmul formulation.

The per-edge attention weight exp(beta * cos(src, dst)) depends only on the
(src, dst) node pair, so the whole message passing collapses to dense algebra:

    G = norm^T norm                  (Gram matrix of L2-normalized features)
    H = C  *  exp(beta * G)          (C = dense dst-by-src edge-count matrix)
    num|den = H^T @ [feat | 1]  ;    out = num / den  rowwise

The count matrix C (dense, from the edge list), the L2-normalized transposed
features, and the [feat | 1] right-hand side are prepared on the host — all
O(N*D) or index work.  The device does the O(N^2 * D) dense work: for each
[128 src x 1024 dst] group, Gram matmuls (PE) -> exp (ACT) -> * C (DVE) ->
accumulating matmuls against [feat|1] (PE), then a rowwise divide.

Sharding: destination nodes are split across the 8 cores; each core computes
its npad/8 output rows end-to-end.  No collectives needed.
"""

import sys
import types

import numpy as np

try:
    from concourse import bacc, mybir, tile
    from concourse.bass_utils import run_bass_kernel_spmd
except ImportError:  # harness container may not have the repo on sys.path
    for _p in ("/opt/trn_rl_repo", "/root/.axon_site/_ro/trn_rl_repo"):
        if _p not in sys.path:
            sys.path.append(_p)
    from concourse import bacc, mybir, tile
    from concourse.bass_utils import run_bass_kernel_spmd

import ml_dtypes

F32 = mybir.dt.float32
BF16 = mybir.dt.bfloat16
AF = mybir.ActivationFunctionType
ALU = mybir.AluOpType

D = 128  # feature dim
GFREE = 1024  # uniform group free size (gm * ck)


def make_cfg(n_nodes=10000, npad=10240, ncores=8, gp_mod=0, schraud_mod=5, lag=2):
    c = types.SimpleNamespace()
    c.n_nodes = n_nodes
    c.npad = npad              # padded node count (multiple of 128*ncores)
    c.ncores = ncores
    c.npc = npad // ncores     # dst columns per core
    c.mch = npad // 128        # source-node chunks (contraction dim)
    c.tt = c.npc // 128        # output row-tiles per core
    c.gp_mod = gp_mod          # 1/gp_mod of C-multiplies routed to GpSimd
    c.schraud_mod = schraud_mod  # 1/schraud_mod of exps via DVE bit-trick
    c.lag = lag                # groups of G->B software pipelining
    # j-chunks of dst columns: prefer 512 wide, remainder in one chunk
    c.jchunks = []             # (joff, ck, gm, ngroups_j)
    off = 0
    while off < c.npc:
        ck = min(512, c.npc - off)
        assert ck % 128 == 0 and GFREE % ck == 0
        gm = GFREE // ck
        assert c.mch % gm == 0
        c.jchunks.append((off, ck, gm, c.mch * ck // GFREE))
        off += ck
    c.ngroups = c.mch * c.npc // GFREE
    return c


def build(cfg):
    """Build the per-core SPMD graph (identical on all cores; data differs)."""
    nc = bacc.Bacc(
        "TRN2", target_bir_lowering=False, debug=False, num_devices=cfg.ncores
    )
    D1 = D + 1
    ntd = nc.dram_tensor("normT", [128, cfg.npad], BF16, kind="ExternalInput")
    nmd = nc.dram_tensor("normTmy", [128, cfg.npc], BF16, kind="ExternalInput")
    fqd = nc.dram_tensor("featq", [128, cfg.mch * D1], BF16, kind="ExternalInput")
    ctd = nc.dram_tensor("ct", [128, cfg.mch * cfg.npc], BF16, kind="ExternalInput")
    outd = nc.dram_tensor("out", [128, cfg.tt, D], F32, kind="ExternalOutput")

    with tile.TileContext(nc) as tc:
        with (
            tc.tile_pool(name="const", bufs=1) as constp,
            tc.tile_pool(name="big", bufs=1) as bigp,
            tc.tile_pool(name="cb", bufs=5) as cbp,
            tc.tile_pool(name="eg", bufs=4) as egp,
            tc.tile_pool(name="ht", bufs=6) as htp,
            tc.tile_pool(name="pg", bufs=3, space="PSUM") as pgp,
            tc.tile_pool(name="po", bufs=2, space="PSUM") as pop,
        ):
            normT = bigp.tile([128, cfg.npad], BF16)
            normTmy = bigp.tile([128, cfg.npc], BF16)
            featq = bigp.tile([128, cfg.mch * D1], BF16)
            outacc = bigp.tile([128, cfg.tt, D1], F32)
            final = bigp.tile([128, cfg.npc], F32)

            nc.sync.dma_start(normTmy[:], nmd[:])
            nsplit = 8
            stepn = max(128, (cfg.npad // nsplit) // 128 * 128)
            stepq = max(D1, (cfg.mch * D1 // nsplit) // D1 * D1)
            qoffs = list(range(0, cfg.mch * D1, stepq))
            noffs = list(range(0, cfg.npad, stepn))
            cb_tiles = {}  # super-tile pair index -> (tile, served)

            def fetch_cb_pair(pix):
                # one DMA covers two consecutive groups (bigger transfers
                # sustain higher DMA bandwidth)
                lo = pix * 2 * GFREE
                hi = min((pix * 2 + 2) * GFREE, cfg.mch * cfg.npc)
                cbt = cbp.tile([128, 2 * GFREE], BF16, tag="cb", name="cb")
                nc.sync.dma_start(cbt[:, 0 : hi - lo], ctd[:, lo:hi])
                cb_tiles[pix] = cbt

            def get_cb(g):
                pix = g // 2
                if pix not in cb_tiles:
                    fetch_cb_pair(pix)
                return cb_tiles[pix][:, (g % 2) * GFREE : (g % 2 + 1) * GFREE]

            npair = (cfg.ngroups + 1) // 2
            for ix in range(max(len(qoffs), len(noffs))):
                if ix < min(3, npair):  # stream C from t=0
                    fetch_cb_pair(ix)
                if ix < len(noffs):
                    a = noffs[ix]
                    b = min(a + stepn, cfg.npad)
                    nc.sync.dma_start(normT[:, a:b], ntd[:, a:b])
                if ix < len(qoffs):
                    a = qoffs[ix]
                    b = min(a + stepq, cfg.mch * D1)
                    nc.sync.dma_start(featq[:, a:b], fqd[:, a:b])

            dmax = constp.tile([128, cfg.tt], F32)
            rden = constp.tile([128, cfg.tt], F32)

            # ---- main loop: uniform [128, GFREE] groups, software-
            # pipelined so a group's B-matmuls trail its G-matmuls by
            # cfg.lag groups (hides the psum->exp->mult latency on PE) ----
            groups = []  # flat (joff, ck, gm, ns, gj, ngj, jix)
            for jix, (joff, ck, gm, ngj) in enumerate(cfg.jchunks):
                for gj in range(ngj):
                    groups.append((joff, ck, gm, ck // 128, gj, ngj, jix))
            po_by_j = {}
            ht_by_g = {}

            def emit_front(gidx):
                joff, ck, gm, ns, gj, ngj, jix = groups[gidx]
                if jix not in po_by_j:
                    po_by_j[jix] = [
                        pop.tile([128, 2 * D1], F32, tag="po", name=f"po{jix}_{u}")
                        for u in range((ns + 1) // 2)
                    ]
                pg = pgp.tile([128, GFREE], F32, tag="pg", name="pg")
                for k in range(gm):
                    i = gj * gm + k
                    nc.tensor.matmul(
                        pg[:, k * ck : (k + 1) * ck],
                        normT[:, i * 128 : (i + 1) * 128],
                        normTmy[:, joff : joff + ck],
                        start=True, stop=True,
                    )
                cb = get_cb(gidx)
                mult_eng = (
                    nc.gpsimd
                    if cfg.gp_mod and (gidx % cfg.gp_mod) == 2
                    else nc.vector
                )
                ht = htp.tile([128, GFREE], BF16, tag="ht", name="ht")
                if cfg.schraud_mod and (gidx % cfg.schraud_mod) == 1:
                    # exp(x) ~= bf16_bits(round(184.665*x + 16250.4)):
                    # Schraudolph bit-trick on DVE, offloading ScalarE
                    si = egp.tile(
                        [128, GFREE], mybir.dt.int16, tag="eg", name="si"
                    )
                    nc.vector.tensor_scalar(
                        out=si[:], in0=pg[:], scalar1=184.664965,
                        scalar2=16250.4, op0=ALU.mult, op1=ALU.add,
                    )
                    mult_eng.tensor_tensor(
                        ht[:], si[:].bitcast(BF16), cb, op=ALU.mult
                    )
                    cb_tiles.pop(gidx // 2, None) if gidx % 2 else None
                else:
                    eg = egp.tile([128, GFREE], BF16, tag="eg", name="eg")
                    nc.scalar.activation(eg[:], pg[:], AF.Exp)
                    mult_eng.tensor_tensor(ht[:], eg[:], cb, op=ALU.mult)
                    cb_tiles.pop(gidx // 2, None) if gidx % 2 else None
                ht_by_g[gidx] = ht

            def emit_back(gidx):
                joff, ck, gm, ns, gj, ngj, jix = groups[gidx]
                ht = ht_by_g.pop(gidx)
                po = po_by_j[jix]
                for k in range(gm):
                    i = gj * gm + k
                    for s in range(ns):
                        nc.tensor.matmul(
                            po[s // 2][:, (s % 2) * D1 : (s % 2 + 1) * D1],
                            ht[:, k * ck + s * 128 : k * ck + (s + 1) * 128],
                            featq[:, i * D1 : (i + 1) * D1],
                            # start zeroes the whole 2KB PSUM bank, so only
                            # the first region of each packed pair sets it
                            start=(gj == 0 and k == 0 and s % 2 == 0),
                            stop=(gj == ngj - 1 and k == gm - 1),
                            skip_group_check=True,
                        )
                if gj == ngj - 1:  # last group of this j-chunk: drain po,
                    t0 = joff // 128   # divide and ship this slice out now
                    for s in range(ns):
                        nc.vector.tensor_copy(
                            outacc[:, t0 + s, :],
                            po[s // 2][:, (s % 2) * D1 : (s % 2 + 1) * D1],
                        )
                    nc.vector.tensor_scalar(
                        out=dmax[:, t0 : t0 + ns],
                        in0=outacc[:, t0 : t0 + ns, D : D + 1],
                        scalar1=1e-30, scalar2=None, op0=ALU.max,
                    )
                    nc.vector.reciprocal(
                        rden[:, t0 : t0 + ns], dmax[:, t0 : t0 + ns]
                    )
                    for s in range(ns):
                        t = t0 + s
                        nc.vector.tensor_scalar(
                            out=final[:, t * D : (t + 1) * D],
                            in0=outacc[:, t, 0:D],
                            scalar1=rden[:, t : t + 1], scalar2=None,
                            op0=ALU.mult,
                        )
                    nc.sync.dma_start(
                        outd[:, t0 : t0 + ns, :],
                        final[:, t0 * D : (t0 + ns) * D].rearrange(
                            "p (t d) -> p t d", d=D
                        ),
                    )

            for g in range(cfg.ngroups + cfg.lag):
                if g < cfg.ngroups:
                    emit_front(g)
                if g >= cfg.lag:
                    emit_back(g - cfg.lag)

    nc.compile()
    return nc


def prepare_inputs(feat, src, dst, beta, cfg):
    feat = np.ascontiguousarray(np.asarray(feat), dtype=np.float32)
    src = np.asarray(src).astype(np.int64)
    dst = np.asarray(dst).astype(np.int64)
    beta = np.asarray(beta, dtype=np.float32).reshape(-1)
    D1 = D + 1

    featp = np.zeros((cfg.npad, D), np.float32)
    featp[: cfg.n_nodes] = feat
    rn = 1.0 / np.maximum(np.linalg.norm(featp, axis=1, keepdims=True), 1e-12)
    normp = featp * rn
    normT = np.ascontiguousarray(normp.T.astype(ml_dtypes.bfloat16))  # [128,npad]

    # featq: [128, mch*(D+1)] bf16; block i col D holds the bias 1.0
    fq = np.ones((128, cfg.mch, D1), dtype=ml_dtypes.bfloat16)
    fq[:, :, :D] = (
        featp.astype(ml_dtypes.bfloat16).reshape(cfg.mch, 128, D).transpose(1, 0, 2)
    )
    fq = np.ascontiguousarray(fq.reshape(128, cfg.mch * D1))

    in_maps = []
    for c in range(cfg.ncores):
        lo = c * cfg.npc
        nmy = np.ascontiguousarray(
            (beta[0] * normp[lo : lo + cfg.npc]).T.astype(ml_dtypes.bfloat16)
        )
        m = (dst >= lo) & (dst < lo + cfg.npc)
        s_c = src[m]
        d_c = dst[m] - lo
        cnt = np.bincount(
            s_c * cfg.npc + d_c, minlength=cfg.npad * cfg.npc
        ).reshape(cfg.npad, cfg.npc)
        # group-major C layout: per j-chunk, per group: [128, gm*ck]
        blocks = []
        for joff, ck, gm, ngj in cfg.jchunks:
            blk = cnt[:, joff : joff + ck].reshape(ngj, gm, 128, ck)
            blocks.append(blk.transpose(2, 0, 1, 3).reshape(128, ngj * gm * ck))
        ct = np.ascontiguousarray(
            np.concatenate(blocks, axis=1).astype(ml_dtypes.bfloat16)
        )
        in_maps.append(
            {"normT": normT, "normTmy": nmy, "featq": fq, "ct": ct}
        )
    return in_maps


def postprocess(results, cfg):
    parts = []
    for c in range(cfg.ncores):
        o = np.asarray(results[c]["out"], np.float32)  # [128, tt, D]
        parts.append(o.transpose(1, 0, 2).reshape(cfg.npc, D))
    return np.concatenate(parts, axis=0)[: cfg.n_nodes]


_CACHE = {}


def _get_nc(cfg):
    key = (cfg.npad, cfg.ncores, cfg.gp_mod, cfg.schraud_mod, cfg.lag)
    if key not in _CACHE:
        _CACHE[key] = build(cfg)
    return _CACHE[key]


def kernel(feat, src, dst, beta):
    cfg = make_cfg()
    nc = _get_nc(cfg)
    in_maps = prepare_inputs(feat, src, dst, beta, cfg)
    res = run_bass_kernel_spmd(nc, in_maps, core_ids=list(range(cfg.ncores)))
    return postprocess(res.results, cfg)
